# revision 3
# baseline (speedup 1.0000x reference)
"""APNB (asymmetric pyramid non-local block) sparse-attention kernel for 8 TRN2 NeuronCores.

Strategy: pure data-parallel over batch (B=8 -> one batch element per core, no
collectives). Per core, the whole block is computed with bf16 TensorE GEMMs
(f32 PSUM accumulation):

  host folds:  BN+bias into conv weights; W (value->out conv) folded into the
               first half of the output conv:  Wc = (so*Wo1) @ Ww.
  device:      KF  = relu(Wk' @ X + bk')                 (256, 9216)
               G24 = 4x4-block sums of X and KF          (DVE reduce)
               PF/KP = PSP pyramid (1,3,6,8) means       (DVE, from G24)
               VT  = PF^T @ Wv^T                         (110, 256)
               SIM^T = KP^T @ KF                         (110, N) per tile
               attn = exp(s*SIM - ln(colsum))            (div-free softmax,
                      ln folded back into PSUM via a rank-1 matmul)
               CTX^T = VT^T @ attn                       (256, N)
               OUT = relu(Wc @ CTX^T + Bw' @ X + b')     (512, N)

The softmax is computed without max-subtraction: |s*sim| is O(1) for this
problem's data distribution, so exp is safe in f32.
"""

import numpy as np
import ml_dtypes
from contextlib import ExitStack

import concourse.bass as bass
import concourse.bacc as bacc
import concourse.mybir as mybir
import concourse.tile as tile
from concourse.bass import ts
from concourse.bass_utils import run_bass_kernel_spmd

P = 128
CIN, CK, CV, COUT = 512, 256, 256, 512
H = W = 96
N = H * W              # 9216
NT = 512               # matmul free-dim tile
NTILES = N // NT       # 18
S = 110                # pooled tokens: 1+9+36+64
EPS = 1e-5
F32 = mybir.dt.float32
BF16 = mybir.dt.bfloat16
RELU = mybir.ActivationFunctionType.Relu
EXP = mybir.ActivationFunctionType.Exp
LN = mybir.ActivationFunctionType.Ln
AX = mybir.AxisListType

N_CORES = 8

# set by test harness to capture profile info
PROFILE = False
LAST_RESULT = None

_NC = None


def _psp_stage2(nc, pool, g24, ko, pf):
    """g24: [P, ko, 24, 24] f32 holding 4x4-pixel SUMS. pf: [P, ko, 110] bf16 out.

    Emits the pyramid (1, 3, 6, 8) means in reference concat order.
    Scale-6 cells are 4x4 grid cells; scale-8 cells are 3x3; scale-3 = 2x2 of
    scale-6; scale-1 = sum of all scale-3.
    """
    f = F32
    # ---- scale 6 (cells of 4x4 grid entries = 16x16 px) ----
    c6 = pool.tile([P, ko, 24, 6], f, tag="c6")
    nc.vector.reduce_sum(c6, g24.rearrange("p k a (b bi) -> p k a b bi", bi=4), axis=AX.X)
    v6 = c6.rearrange("p k (a ai) b -> p k a ai b", ai=4)
    s6 = pool.tile([P, ko, 6, 6], f, tag="s6")
    nc.vector.tensor_add(s6, v6[:, :, :, 0, :], v6[:, :, :, 1, :])
    nc.vector.tensor_add(s6, s6, v6[:, :, :, 2, :])
    nc.vector.tensor_add(s6, s6, v6[:, :, :, 3, :])
    nc.vector.tensor_scalar_mul(
        pf[:, :, 10:46], s6.rearrange("p k a b -> p k (a b)"), 1.0 / 256.0
    )
    # ---- scale 3 (2x2 of scale-6 cells = 32x32 px) ----
    c3 = pool.tile([P, ko, 6, 3], f, tag="c3")
    nc.vector.reduce_sum(c3, s6.rearrange("p k a (b bi) -> p k a b bi", bi=2), axis=AX.X)
    v3 = c3.rearrange("p k (a ai) b -> p k a ai b", ai=2)
    s3 = pool.tile([P, ko, 3, 3], f, tag="s3")
    nc.vector.tensor_add(s3, v3[:, :, :, 0, :], v3[:, :, :, 1, :])
    nc.vector.tensor_scalar_mul(
        pf[:, :, 1:10], s3.rearrange("p k a b -> p k (a b)"), 1.0 / 1024.0
    )
    # ---- scale 1 ----
    t1 = pool.tile([P, ko, 1], f, tag="t1")
    nc.vector.reduce_sum(t1, s3.rearrange("p k a b -> p k (a b)"), axis=AX.X)
    nc.vector.tensor_scalar_mul(pf[:, :, 0:1], t1, 1.0 / 9216.0)
    # ---- scale 8 (cells of 3x3 grid entries = 12x12 px) ----
    c8 = pool.tile([P, ko, 24, 8], f, tag="c8")
    nc.vector.reduce_sum(c8, g24.rearrange("p k a (b bi) -> p k a b bi", bi=3), axis=AX.X)
    v8 = c8.rearrange("p k (a ai) b -> p k a ai b", ai=3)
    s8 = pool.tile([P, ko, 8, 8], f, tag="s8")
    nc.vector.tensor_add(s8, v8[:, :, :, 0, :], v8[:, :, :, 1, :])
    nc.vector.tensor_add(s8, s8, v8[:, :, :, 2, :])
    nc.vector.tensor_scalar_mul(
        pf[:, :, 46:110], s8.rearrange("p k a b -> p k (a b)"), 1.0 / 144.0
    )


def _build_body(ctx: ExitStack, tc: tile.TileContext, x_d, wkt_d, wvt_d, wct_d,
                bwt_d, bk_d, bf_d, out_d):
    nc = tc.nc

    consts = ctx.enter_context(tc.tile_pool(name="consts", bufs=1))
    big = ctx.enter_context(tc.tile_pool(name="big", bufs=1))
    stage = ctx.enter_context(tc.tile_pool(name="stage", bufs=3))
    poolb = ctx.enter_context(tc.tile_pool(name="poolb", bufs=1))
    work = ctx.enter_context(tc.tile_pool(name="work", bufs=2))
    outp = ctx.enter_context(tc.tile_pool(name="outp", bufs=4))
    psum = ctx.enter_context(tc.tile_pool(name="psum", bufs=1, space="PSUM"))

    # ---- weights / constants into SBUF ----
    wkt = consts.tile([P, 4, CK], BF16)
    nc.sync.dma_start(wkt, wkt_d[:].rearrange("(kc p) m -> p kc m", p=P))
    wvt = consts.tile([P, 4, CV], BF16)
    nc.sync.dma_start(wvt, wvt_d[:].rearrange("(kc p) m -> p kc m", p=P))
    wct = consts.tile([P, 2, COUT], BF16)
    nc.sync.dma_start(wct, wct_d[:].rearrange("(kc p) m -> p kc m", p=P))
    bwt = consts.tile([P, 4, COUT], BF16)
    nc.sync.dma_start(bwt, bwt_d[:].rearrange("(kc p) m -> p kc m", p=P))
    bkb = consts.tile([P, 2], F32)
    nc.sync.dma_start(bkb, bk_d[:].rearrange("(mc p) -> p mc", p=P))
    bfb = consts.tile([P, 4], F32)
    nc.sync.dma_start(bfb, bf_d[:].rearrange("(mc p) -> p mc", p=P))
    ones_col = consts.tile([S, 1], BF16)
    nc.vector.memset(ones_col, 1.0)
    negs = consts.tile([1, S], BF16)
    nc.vector.memset(negs, -16.0)

    # ---- persistent full-res activations (bf16) ----
    xb = big.tile([P, 4, N], BF16)    # X, channels on partitions (4 chunks)
    kfb = big.tile([P, 2, N], BF16)   # relu(key/query features)

    xv = x_d[:].rearrange("(kc p) n -> p kc n", p=P)
    ov = out_d[:].rearrange("(mc p) n -> p mc n", p=P)

    # ---- phase 1: load/cast X, KF = relu(Wk' @ X + bk') ----
    for t in range(NTILES):
        xs = stage.tile([P, 4, NT], F32, tag="xs")
        nc.sync.dma_start(xs, xv[:, :, ts(t, NT)])
        nc.gpsimd.tensor_copy(out=xb[:, :, ts(t, NT)], in_=xs)
        for mc in range(2):
            ps = psum.tile([P, NT], F32, tag="big_ps")
            for kc in range(4):
                nc.tensor.matmul(
                    ps, wkt[:, kc, ts(mc, P)], xb[:, kc, ts(t, NT)],
                    start=(kc == 0), stop=(kc == 3),
                )
            nc.scalar.activation(kfb[:, mc, ts(t, NT)], ps, RELU,
                                 bias=bkb[:, mc:mc + 1])

    # ---- pooling stage 1: 4x4-pixel block sums -> 24x24 grids ----
    g24x = poolb.tile([P, 4, 24, 24], F32)
    for k in range(4):
        xk = xb[:, k, :].rearrange("p (h w) -> p h w", w=W)
        for hc in range(8):
            src = xk[:, ts(hc, 12), :].rearrange(
                "p (hb hi) (wb wi) -> p hb wb hi wi", hi=4, wi=4)
            nc.vector.reduce_sum(g24x[:, k, ts(hc, 3), :], src, axis=AX.XY)
    g24k = poolb.tile([P, 2, 24, 24], F32)
    for k in range(2):
        kk = kfb[:, k, :].rearrange("p (h w) -> p h w", w=W)
        for hc in range(8):
            src = kk[:, ts(hc, 12), :].rearrange(
                "p (hb hi) (wb wi) -> p hb wb hi wi", hi=4, wi=4)
            nc.vector.reduce_sum(g24k[:, k, ts(hc, 3), :], src, axis=AX.XY)

    # ---- pooling stage 2: pyramid means ----
    pfx = consts.tile([P, 4, S], BF16)   # pooled X      (CIN on partitions)
    kpx = consts.tile([P, 2, S], BF16)   # pooled KF     (CK on partitions)
    _psp_stage2(nc, poolb, g24x, 4, pfx)
    _psp_stage2(nc, poolb, g24k, 2, kpx)

    # ---- VT = PF^T @ Wv^T : (110, 256) ----
    vt_ps = psum.tile([P, CV], F32, tag="sim_ps")
    for kc in range(4):
        nc.tensor.matmul(vt_ps[:S, :], pfx[:, kc, :], wvt[:, kc, :],
                         start=(kc == 0), stop=(kc == 3))
    vt = consts.tile([S, CV], BF16)
    nc.scalar.copy(vt, vt_ps[:S, :])

    # ---- phase 3: attention + output, streamed over N tiles ----
    for t in range(NTILES):
        sim_ps = psum.tile([P, NT], F32, tag="sim_ps")
        nc.tensor.matmul(sim_ps[:S, :], kpx[:, 0, :], kfb[:, 0, ts(t, NT)],
                         start=True, stop=False)
        nc.tensor.matmul(sim_ps[:S, :], kpx[:, 1, :], kfb[:, 1, ts(t, NT)],
                         start=False, stop=True)
        e1 = work.tile([P, NT], BF16, tag="e1")
        nc.scalar.activation(e1[:S, :], sim_ps[:S, :], EXP, scale=0.0625)
        cs_ps = psum.tile([1, NT], F32, tag="aux_ps")
        nc.tensor.matmul(cs_ps, ones_col, e1[:S, :], start=True, stop=True)
        lrow = work.tile([1, NT], BF16, tag="lrow")
        nc.scalar.activation(lrow, cs_ps, LN)
        # rank-1 update: sim += (-16) * ln(colsum), so exp(sim/16) is normalized
        nc.tensor.matmul(sim_ps[:S, :], negs, lrow, start=False, stop=True,
                         skip_group_check=True)
        en = work.tile([P, NT], BF16, tag="en")
        nc.scalar.activation(en[:S, :], sim_ps[:S, :], EXP, scale=0.0625)

        ctxb = work.tile([P, 2, NT], BF16, tag="ctxb")
        for vc in range(2):
            ctx_ps = psum.tile([P, NT], F32, tag="aux_ps")
            nc.tensor.matmul(ctx_ps, vt[:, ts(vc, P)], en[:S, :],
                             start=True, stop=True)
            nc.scalar.copy(ctxb[:, vc, :], ctx_ps)

        for mc in range(4):
            out_ps = psum.tile([P, NT], F32, tag="big_ps")
            nc.tensor.matmul(out_ps, wct[:, 0, ts(mc, P)], ctxb[:, 0, :],
                             start=True, stop=False)
            nc.tensor.matmul(out_ps, wct[:, 1, ts(mc, P)], ctxb[:, 1, :],
                             start=False, stop=False)
            for kc in range(4):
                nc.tensor.matmul(out_ps, bwt[:, kc, ts(mc, P)],
                                 xb[:, kc, ts(t, NT)],
                                 start=False, stop=(kc == 3))
            osb = outp.tile([P, NT], F32, tag="osb")
            nc.scalar.activation(osb, out_ps, RELU, bias=bfb[:, mc:mc + 1])
            nc.sync.dma_start(ov[:, mc, ts(t, NT)], osb)


def build_nc():
    nc = bacc.Bacc("TRN2", target_bir_lowering=False, debug=False)
    x_d = nc.declare_dram_parameter("x", [CIN, N], F32, isOutput=False)
    wkt_d = nc.declare_dram_parameter("wkt", [CIN, CK], BF16, isOutput=False)
    wvt_d = nc.declare_dram_parameter("wvt", [CIN, CV], BF16, isOutput=False)
    wct_d = nc.declare_dram_parameter("wct", [CV, COUT], BF16, isOutput=False)
    bwt_d = nc.declare_dram_parameter("bwt", [CIN, COUT], BF16, isOutput=False)
    bk_d = nc.declare_dram_parameter("bk", [CK], F32, isOutput=False)
    bf_d = nc.declare_dram_parameter("bf", [COUT], F32, isOutput=False)
    out_d = nc.declare_dram_parameter("out", [COUT, N], F32, isOutput=True)
    with tile.TileContext(nc) as tc:
        with ExitStack() as ctx:
            _build_body(ctx, tc, x_d, wkt_d, wvt_d, wct_d, bwt_d, bk_d, bf_d,
                        out_d)
    nc.compile()
    return nc


def _get_nc():
    global _NC
    if _NC is None:
        _NC = build_nc()
    return _NC


def fold_params(Wk, bk, gk, betak, mk, vk, Wv, bv, Ww, bw, Wo, bo, go, betao,
                mo, vo):
    """Fold BN params + the Ww conv into effective weights (all f32 numpy)."""
    bf16 = ml_dtypes.bfloat16
    sk = gk / np.sqrt(vk + EPS)
    Wk_f = sk[:, None] * Wk
    bk_f = (bk - mk) * sk + betak
    so = go / np.sqrt(vo + EPS)
    A = so[:, None] * Wo[:, :CIN]      # applies to ctx2 = Ww@ctx + bw
    Bw = so[:, None] * Wo[:, CIN:]     # applies to feats
    b0 = (bo - mo) * so + betao
    Wc = A @ Ww                        # (COUT, CV)
    # attn rows sum to 1  =>  value bias bv contributes Wc @ bv everywhere
    bf_ = b0 + A @ bw + Wc @ bv
    return {
        "wkt": np.ascontiguousarray(Wk_f.T).astype(bf16),
        "wvt": np.ascontiguousarray(Wv.T).astype(bf16),
        "wct": np.ascontiguousarray(Wc.T).astype(bf16),
        "bwt": np.ascontiguousarray(Bw.T).astype(bf16),
        "bk": bk_f.astype(np.float32),
        "bf": bf_.astype(np.float32),
    }


def kernel(**inputs):
    global LAST_RESULT
    feats = np.asarray(inputs["feats"], np.float32)
    B = feats.shape[0]
    assert feats.shape == (B, CIN, H, W) and B == N_CORES

    common = fold_params(
        np.asarray(inputs["Wk"], np.float32), np.asarray(inputs["bk"], np.float32),
        np.asarray(inputs["gk"], np.float32), np.asarray(inputs["betak"], np.float32),
        np.asarray(inputs["mk"], np.float32), np.asarray(inputs["vk"], np.float32),
        np.asarray(inputs["Wv"], np.float32), np.asarray(inputs["bv"], np.float32),
        np.asarray(inputs["Ww"], np.float32), np.asarray(inputs["bw"], np.float32),
        np.asarray(inputs["Wo"], np.float32), np.asarray(inputs["bo"], np.float32),
        np.asarray(inputs["go"], np.float32), np.asarray(inputs["betao"], np.float32),
        np.asarray(inputs["mo"], np.float32), np.asarray(inputs["vo"], np.float32),
    )
    in_maps = [
        {"x": np.ascontiguousarray(feats[i].reshape(CIN, N)), **common}
        for i in range(N_CORES)
    ]
    nc = _get_nc()
    res = run_bass_kernel_spmd(nc, in_maps, core_ids=list(range(N_CORES)),
                               trace=PROFILE)
    LAST_RESULT = res
    out = np.stack([res.results[i]["out"].reshape(COUT, H, W)
                    for i in range(N_CORES)])
    return out.astype(np.float32)


# revision 5
# speedup vs baseline: 1.2096x; 1.2096x over previous
"""APNB (asymmetric pyramid non-local block) sparse-attention kernel for 8 TRN2 NeuronCores.

Strategy: pure data-parallel over batch (B=8 -> one batch element per core, no
collectives). Per core, the whole block is computed with bf16 TensorE GEMMs
(f32 PSUM accumulation):

  host folds:  BN+bias into conv weights; W (value->out conv) folded into the
               first half of the output conv:  Wc = (so*Wo1) @ Ww.
  device:      KF  = relu(Wk' @ X + bk')                 (256, 9216)
               G24 = 4x4-block sums of X and KF          (DVE reduce)
               PF/KP = PSP pyramid (1,3,6,8) means       (DVE, from G24)
               VT  = PF^T @ Wv^T                         (110, 256)
               SIM^T = KP^T @ KF                         (110, N) per tile
               attn = exp(s*SIM - ln(colsum))            (div-free softmax,
                      ln folded back into PSUM via a rank-1 matmul)
               CTX^T = VT^T @ attn                       (256, N)
               OUT = relu(Wc @ CTX^T + Bw' @ X + b')     (512, N)

The softmax is computed without max-subtraction: |s*sim| is O(1) for this
problem's data distribution, so exp is safe in f32.
"""

import numpy as np
import ml_dtypes
from contextlib import ExitStack

import concourse.bass as bass
import concourse.bacc as bacc
import concourse.mybir as mybir
import concourse.tile as tile
from concourse.bass import ts
from concourse.bass_utils import run_bass_kernel_spmd

P = 128
CIN, CK, CV, COUT = 512, 256, 256, 512
H = W = 96
N = H * W              # 9216
NT = 512               # matmul free-dim tile
NTILES = N // NT       # 18
S = 110                # pooled tokens: 1+9+36+64
EPS = 1e-5
F32 = mybir.dt.float32
BF16 = mybir.dt.bfloat16
RELU = mybir.ActivationFunctionType.Relu
EXP = mybir.ActivationFunctionType.Exp
LN = mybir.ActivationFunctionType.Ln
AX = mybir.AxisListType

N_CORES = 8

# set by test harness to capture profile info
PROFILE = False
LAST_RESULT = None

_NC = None


def _psp_stage2(nc, pool, g24, ko, pf):
    """g24: [P, ko, 24, 24] f32 holding 4x4-pixel SUMS. pf: [P, ko, 110] bf16 out.

    Emits the pyramid (1, 3, 6, 8) means in reference concat order.
    Scale-6 cells are 4x4 grid cells; scale-8 cells are 3x3; scale-3 = 2x2 of
    scale-6; scale-1 = sum of all scale-3.
    """
    f = F32
    # ---- scale 6 (cells of 4x4 grid entries = 16x16 px) ----
    c6 = pool.tile([P, ko, 24, 6], f, tag="c6")
    nc.vector.reduce_sum(c6, g24.rearrange("p k a (b bi) -> p k a b bi", bi=4), axis=AX.X)
    v6 = c6.rearrange("p k (a ai) b -> p k a ai b", ai=4)
    s6 = pool.tile([P, ko, 6, 6], f, tag="s6")
    nc.vector.tensor_add(s6, v6[:, :, :, 0, :], v6[:, :, :, 1, :])
    nc.vector.tensor_add(s6, s6, v6[:, :, :, 2, :])
    nc.vector.tensor_add(s6, s6, v6[:, :, :, 3, :])
    nc.vector.tensor_scalar_mul(
        pf[:, :, 10:46], s6.rearrange("p k a b -> p k (a b)"), 1.0 / 256.0
    )
    # ---- scale 3 (2x2 of scale-6 cells = 32x32 px) ----
    c3 = pool.tile([P, ko, 6, 3], f, tag="c3")
    nc.vector.reduce_sum(c3, s6.rearrange("p k a (b bi) -> p k a b bi", bi=2), axis=AX.X)
    v3 = c3.rearrange("p k (a ai) b -> p k a ai b", ai=2)
    s3 = pool.tile([P, ko, 3, 3], f, tag="s3")
    nc.vector.tensor_add(s3, v3[:, :, :, 0, :], v3[:, :, :, 1, :])
    nc.vector.tensor_scalar_mul(
        pf[:, :, 1:10], s3.rearrange("p k a b -> p k (a b)"), 1.0 / 1024.0
    )
    # ---- scale 1 ----
    t1 = pool.tile([P, ko, 1], f, tag="t1")
    nc.vector.reduce_sum(t1, s3.rearrange("p k a b -> p k (a b)"), axis=AX.X)
    nc.vector.tensor_scalar_mul(pf[:, :, 0:1], t1, 1.0 / 9216.0)
    # ---- scale 8 (cells of 3x3 grid entries = 12x12 px) ----
    c8 = pool.tile([P, ko, 24, 8], f, tag="c8")
    nc.vector.reduce_sum(c8, g24.rearrange("p k a (b bi) -> p k a b bi", bi=3), axis=AX.X)
    v8 = c8.rearrange("p k (a ai) b -> p k a ai b", ai=3)
    s8 = pool.tile([P, ko, 8, 8], f, tag="s8")
    nc.vector.tensor_add(s8, v8[:, :, :, 0, :], v8[:, :, :, 1, :])
    nc.vector.tensor_add(s8, s8, v8[:, :, :, 2, :])
    nc.vector.tensor_scalar_mul(
        pf[:, :, 46:110], s8.rearrange("p k a b -> p k (a b)"), 1.0 / 144.0
    )


def _build_body(ctx: ExitStack, tc: tile.TileContext, x_d, wkt_d, wvt_d, wct_d,
                bwt_d, bk_d, bf_d, out_d):
    nc = tc.nc

    consts = ctx.enter_context(tc.tile_pool(name="consts", bufs=1))
    big = ctx.enter_context(tc.tile_pool(name="big", bufs=1))
    stage = ctx.enter_context(tc.tile_pool(name="stage", bufs=3))
    poolb = ctx.enter_context(tc.tile_pool(name="poolb", bufs=1))
    work = ctx.enter_context(tc.tile_pool(name="work", bufs=2))
    outp = ctx.enter_context(tc.tile_pool(name="outp", bufs=4))
    psum = ctx.enter_context(tc.tile_pool(name="psum", bufs=1, space="PSUM"))

    # ---- weights / constants into SBUF ----
    wkt = consts.tile([P, 4, CK], BF16)
    nc.sync.dma_start(wkt, wkt_d[:].rearrange("(kc p) m -> p kc m", p=P))
    wvt = consts.tile([P, 4, CV], BF16)
    nc.sync.dma_start(wvt, wvt_d[:].rearrange("(kc p) m -> p kc m", p=P))
    wct = consts.tile([P, 2, COUT], BF16)
    nc.sync.dma_start(wct, wct_d[:].rearrange("(kc p) m -> p kc m", p=P))
    bwt = consts.tile([P, 4, COUT], BF16)
    nc.sync.dma_start(bwt, bwt_d[:].rearrange("(kc p) m -> p kc m", p=P))
    bkb = consts.tile([P, 2], F32)
    nc.sync.dma_start(bkb, bk_d[:].rearrange("(mc p) -> p mc", p=P))
    bfb = consts.tile([P, 4], F32)
    nc.sync.dma_start(bfb, bf_d[:].rearrange("(mc p) -> p mc", p=P))
    ones_col = consts.tile([S, 1], BF16)
    nc.vector.memset(ones_col, 1.0)
    negs = consts.tile([1, S], BF16)
    nc.vector.memset(negs, -16.0)

    # ---- persistent full-res activations (bf16) ----
    xb = big.tile([P, 4, N], BF16)    # X, channels on partitions (4 chunks)
    kfb = big.tile([P, 2, N], BF16)   # relu(key/query features)

    xv = x_d[:].rearrange("(kc p) n -> p kc n", p=P)
    ov = out_d[:].rearrange("(mc p) n -> p mc n", p=P)

    # ---- phase 1: load/cast X, KF = relu(Wk' @ X + bk') ----
    # Pooling stage-1 reduces are emitted interleaved: 12-row chunk hc of the
    # 96-row image covers columns [hc*1152, (hc+1)*1152), i.e. N-tiles up to
    # ceil(1152*(hc+1)/NT) - 1, so chunk hc is emitted right after that tile.
    g24x = poolb.tile([P, 4, 24, 24], F32)
    g24k = poolb.tile([P, 2, 24, 24], F32)
    xg = xb.rearrange("p k (h w) -> p k h w", w=W)
    kg = kfb.rearrange("p k (h w) -> p k h w", w=W)
    hc_done = 0

    def emit_pool_chunks(upto_col):
        nonlocal hc_done
        while hc_done < 8 and (hc_done + 1) * 1152 <= upto_col:
            hc = hc_done
            for k in range(4):
                src = xg[:, k, ts(hc, 12), :].rearrange(
                    "p (hb hi) (wb wi) -> p hb wb hi wi", hi=4, wi=4)
                nc.vector.reduce_sum(g24x[:, k, ts(hc, 3), :], src, axis=AX.XY)
            for k in range(2):
                src = kg[:, k, ts(hc, 12), :].rearrange(
                    "p (hb hi) (wb wi) -> p hb wb hi wi", hi=4, wi=4)
                nc.vector.reduce_sum(g24k[:, k, ts(hc, 3), :], src, axis=AX.XY)
            hc_done += 1

    for t in range(NTILES):
        xs = stage.tile([P, 4, NT], F32, tag="xs")
        nc.sync.dma_start(xs, xv[:, :, ts(t, NT)])
        nc.vector.tensor_copy(out=xb[:, :, ts(t, NT)], in_=xs)
        for mc in range(2):
            ps = psum.tile([P, NT], F32, tag="big_ps")
            for kc in range(4):
                nc.tensor.matmul(
                    ps, wkt[:, kc, ts(mc, P)], xb[:, kc, ts(t, NT)],
                    start=(kc == 0), stop=(kc == 3),
                )
            nc.scalar.activation(kfb[:, mc, ts(t, NT)], ps, RELU,
                                 bias=bkb[:, mc:mc + 1])
        emit_pool_chunks((t + 1) * NT)

    # ---- pooling stage 2: pyramid means ----
    pfx = consts.tile([P, 4, S], BF16)   # pooled X      (CIN on partitions)
    kpx = consts.tile([P, 2, S], BF16)   # pooled KF     (CK on partitions)
    _psp_stage2(nc, poolb, g24x, 4, pfx)
    _psp_stage2(nc, poolb, g24k, 2, kpx)

    # ---- VT = PF^T @ Wv^T : (110, 256) ----
    vt_ps = psum.tile([P, CV], F32, tag="sim_ps")
    for kc in range(4):
        nc.tensor.matmul(vt_ps[:S, :], pfx[:, kc, :], wvt[:, kc, :],
                         start=(kc == 0), stop=(kc == 3))
    vt = consts.tile([S, CV], BF16)
    nc.scalar.copy(vt, vt_ps[:S, :])

    # ---- phase 3: attention + output, streamed over N tiles ----
    for t in range(NTILES):
        sim_ps = psum.tile([P, NT], F32, tag="sim_ps")
        nc.tensor.matmul(sim_ps[:S, :], kpx[:, 0, :], kfb[:, 0, ts(t, NT)],
                         start=True, stop=False)
        nc.tensor.matmul(sim_ps[:S, :], kpx[:, 1, :], kfb[:, 1, ts(t, NT)],
                         start=False, stop=True)
        e1 = work.tile([P, NT], BF16, tag="e1")
        nc.scalar.activation(e1[:S, :], sim_ps[:S, :], EXP, scale=0.0625)
        cs_ps = psum.tile([1, NT], F32, tag="aux_ps")
        nc.tensor.matmul(cs_ps, ones_col, e1[:S, :], start=True, stop=True)
        lrow = work.tile([1, NT], BF16, tag="lrow")
        nc.scalar.activation(lrow, cs_ps, LN)
        # rank-1 update: sim += (-16) * ln(colsum), so exp(sim/16) is normalized
        nc.tensor.matmul(sim_ps[:S, :], negs, lrow, start=False, stop=True,
                         skip_group_check=True)
        en = work.tile([P, NT], BF16, tag="en")
        nc.scalar.activation(en[:S, :], sim_ps[:S, :], EXP, scale=0.0625)

        ctxb = work.tile([P, 2, NT], BF16, tag="ctxb")
        for vc in range(2):
            ctx_ps = psum.tile([P, NT], F32, tag="aux_ps")
            nc.tensor.matmul(ctx_ps, vt[:, ts(vc, P)], en[:S, :],
                             start=True, stop=True)
            nc.scalar.copy(ctxb[:, vc, :], ctx_ps)

        for mc in range(4):
            out_ps = psum.tile([P, NT], F32, tag="big_ps")
            nc.tensor.matmul(out_ps, wct[:, 0, ts(mc, P)], ctxb[:, 0, :],
                             start=True, stop=False)
            nc.tensor.matmul(out_ps, wct[:, 1, ts(mc, P)], ctxb[:, 1, :],
                             start=False, stop=False)
            for kc in range(4):
                nc.tensor.matmul(out_ps, bwt[:, kc, ts(mc, P)],
                                 xb[:, kc, ts(t, NT)],
                                 start=False, stop=(kc == 3))
            osb = outp.tile([P, NT], F32, tag="osb")
            nc.scalar.activation(osb, out_ps, RELU, bias=bfb[:, mc:mc + 1])
            nc.sync.dma_start(ov[:, mc, ts(t, NT)], osb)


def _patch_act_tables():
    """Force every activation onto the one table that holds Exp, Ln, Relu and
    Copy together (`natural_log_exp_and_others`), so the kernel does a single
    ACT_TABLE_LOAD instead of reloading on every Exp<->Ln<->Relu switch.

    Table ids are positional (index into act_info.json), so we keep the dict
    order/size and just empty the other entries.
    """
    import concourse.hw_specs as hw_specs

    if getattr(bacc, "_apnb_act_patch", False):
        return
    orig = hw_specs.get_activation_tables

    def patched(module_arch):
        tabs = orig(module_arch)
        keep = "natural_log_exp_and_others"
        if keep not in tabs:
            return tabs
        return {k: (v if k == keep else set()) for k, v in tabs.items()}

    bacc.get_activation_tables = patched
    bacc._apnb_act_patch = True


def build_nc():
    _patch_act_tables()
    nc = bacc.Bacc("TRN2", target_bir_lowering=False, debug=False)
    x_d = nc.declare_dram_parameter("x", [CIN, N], F32, isOutput=False)
    wkt_d = nc.declare_dram_parameter("wkt", [CIN, CK], BF16, isOutput=False)
    wvt_d = nc.declare_dram_parameter("wvt", [CIN, CV], BF16, isOutput=False)
    wct_d = nc.declare_dram_parameter("wct", [CV, COUT], BF16, isOutput=False)
    bwt_d = nc.declare_dram_parameter("bwt", [CIN, COUT], BF16, isOutput=False)
    bk_d = nc.declare_dram_parameter("bk", [CK], F32, isOutput=False)
    bf_d = nc.declare_dram_parameter("bf", [COUT], F32, isOutput=False)
    out_d = nc.declare_dram_parameter("out", [COUT, N], F32, isOutput=True)
    with tile.TileContext(nc) as tc:
        with ExitStack() as ctx:
            _build_body(ctx, tc, x_d, wkt_d, wvt_d, wct_d, bwt_d, bk_d, bf_d,
                        out_d)
    nc.compile()
    return nc


def _get_nc():
    global _NC
    if _NC is None:
        _NC = build_nc()
    return _NC


def fold_params(Wk, bk, gk, betak, mk, vk, Wv, bv, Ww, bw, Wo, bo, go, betao,
                mo, vo):
    """Fold BN params + the Ww conv into effective weights (all f32 numpy)."""
    bf16 = ml_dtypes.bfloat16
    sk = gk / np.sqrt(vk + EPS)
    Wk_f = sk[:, None] * Wk
    bk_f = (bk - mk) * sk + betak
    so = go / np.sqrt(vo + EPS)
    A = so[:, None] * Wo[:, :CIN]      # applies to ctx2 = Ww@ctx + bw
    Bw = so[:, None] * Wo[:, CIN:]     # applies to feats
    b0 = (bo - mo) * so + betao
    Wc = A @ Ww                        # (COUT, CV)
    # attn rows sum to 1  =>  value bias bv contributes Wc @ bv everywhere
    bf_ = b0 + A @ bw + Wc @ bv
    return {
        "wkt": np.ascontiguousarray(Wk_f.T).astype(bf16),
        "wvt": np.ascontiguousarray(Wv.T).astype(bf16),
        "wct": np.ascontiguousarray(Wc.T).astype(bf16),
        "bwt": np.ascontiguousarray(Bw.T).astype(bf16),
        "bk": bk_f.astype(np.float32),
        "bf": bf_.astype(np.float32),
    }


def kernel(**inputs):
    global LAST_RESULT
    feats = np.asarray(inputs["feats"], np.float32)
    B = feats.shape[0]
    assert feats.shape == (B, CIN, H, W) and B == N_CORES

    common = fold_params(
        np.asarray(inputs["Wk"], np.float32), np.asarray(inputs["bk"], np.float32),
        np.asarray(inputs["gk"], np.float32), np.asarray(inputs["betak"], np.float32),
        np.asarray(inputs["mk"], np.float32), np.asarray(inputs["vk"], np.float32),
        np.asarray(inputs["Wv"], np.float32), np.asarray(inputs["bv"], np.float32),
        np.asarray(inputs["Ww"], np.float32), np.asarray(inputs["bw"], np.float32),
        np.asarray(inputs["Wo"], np.float32), np.asarray(inputs["bo"], np.float32),
        np.asarray(inputs["go"], np.float32), np.asarray(inputs["betao"], np.float32),
        np.asarray(inputs["mo"], np.float32), np.asarray(inputs["vo"], np.float32),
    )
    in_maps = [
        {"x": np.ascontiguousarray(feats[i].reshape(CIN, N)), **common}
        for i in range(N_CORES)
    ]
    nc = _get_nc()
    res = run_bass_kernel_spmd(nc, in_maps, core_ids=list(range(N_CORES)),
                               trace=PROFILE)
    LAST_RESULT = res
    out = np.stack([res.results[i]["out"].reshape(COUT, H, W)
                    for i in range(N_CORES)])
    return out.astype(np.float32)


# revision 7
# speedup vs baseline: 1.5099x; 1.2482x over previous
"""APNB (asymmetric pyramid non-local block) sparse-attention kernel for 8 TRN2 NeuronCores.

Strategy: pure data-parallel over batch (B=8 -> one batch element per core, no
collectives). Per core, the whole block is computed with bf16 TensorE GEMMs
(f32 PSUM accumulation):

  host folds:  BN+bias into conv weights; W (value->out conv) folded into the
               first half of the output conv:  Wc = (so*Wo1) @ Ww.
  phase 1 (streamed over N tiles, X transient in SBUF):
               KF   = relu(Wk' @ X + bk')          (256, 9216)  persistent bf16
               OUTB = Bw' @ X                      (512, 9216)  persistent bf16
               col-pool partial sums of X and KF   (DVE reduce)
  phase 2:     row-pool + PSP pyramid (1,3,6,8) means -> PF, KP
               VT = PF^T @ Wv^T                    (110, 256)
  phase 3 (streamed):
               SIM^T = KP^T @ KF                   (110, N)
               attn  = exp(s*SIM - ln(colsum))     (div-free softmax; the ln is
                       folded into PSUM via a rank-1 matmul)
               CTX^T = VT^T @ attn                 (256, N)
               OUT   = relu(OUTB + Wc @ CTX^T + b')  (OUTB re-injected into
                       PSUM with an identity matmul)

The softmax needs no max-subtraction: |s*sim| is O(1) for this problem's data
distribution, so exp is safe in f32.
"""

import numpy as np
import ml_dtypes
from contextlib import ExitStack

import concourse.bass as bass
import concourse.bacc as bacc
import concourse.mybir as mybir
import concourse.tile as tile
from concourse.bass import ts
from concourse.bass_utils import run_bass_kernel_spmd
from concourse.masks import make_identity

P = 128
CIN, CK, CV, COUT = 512, 256, 256, 512
H = W = 96
N = H * W              # 9216
NT = 512               # matmul free-dim tile
NTILES = N // NT       # 18
S = 110                # pooled tokens: 1+9+36+64
EPS = 1e-5
F32 = mybir.dt.float32
BF16 = mybir.dt.bfloat16
RELU = mybir.ActivationFunctionType.Relu
EXP = mybir.ActivationFunctionType.Exp
LN = mybir.ActivationFunctionType.Ln
COPY = mybir.ActivationFunctionType.Copy
ADD = mybir.AluOpType.add
MAX = mybir.AluOpType.max
AX = mybir.AxisListType

N_CORES = 8

# set by test harness to capture profile info
PROFILE = False
LAST_RESULT = None

_NC = None


def _psp_stage2(nc, pool, g24, ko, pf):
    """g24: [P, ko, 24, 24] f32 holding 4x4-pixel SUMS. pf: [P, ko, 110] bf16 out.

    Emits the pyramid (1, 3, 6, 8) means in reference concat order.
    Scale-6 cells are 4x4 grid cells; scale-8 cells are 3x3; scale-3 = 2x2 of
    scale-6; scale-1 = sum of all scale-3.
    """
    f = F32
    # ---- scale 6 (cells of 4x4 grid entries = 16x16 px) ----
    c6 = pool.tile([P, ko, 24, 6], f, tag="c6")
    nc.vector.reduce_sum(c6, g24.rearrange("p k a (b bi) -> p k a b bi", bi=4), axis=AX.X)
    v6 = c6.rearrange("p k (a ai) b -> p k a ai b", ai=4)
    s6 = pool.tile([P, ko, 6, 6], f, tag="s6")
    nc.vector.tensor_add(s6, v6[:, :, :, 0, :], v6[:, :, :, 1, :])
    nc.vector.tensor_add(s6, s6, v6[:, :, :, 2, :])
    nc.vector.tensor_add(s6, s6, v6[:, :, :, 3, :])
    nc.vector.tensor_scalar_mul(
        pf[:, :, 10:46], s6.rearrange("p k a b -> p k (a b)"), 1.0 / 256.0
    )
    # ---- scale 3 (2x2 of scale-6 cells = 32x32 px) ----
    c3 = pool.tile([P, ko, 6, 3], f, tag="c3")
    nc.vector.reduce_sum(c3, s6.rearrange("p k a (b bi) -> p k a b bi", bi=2), axis=AX.X)
    v3 = c3.rearrange("p k (a ai) b -> p k a ai b", ai=2)
    s3 = pool.tile([P, ko, 3, 3], f, tag="s3")
    nc.vector.tensor_add(s3, v3[:, :, :, 0, :], v3[:, :, :, 1, :])
    nc.vector.tensor_scalar_mul(
        pf[:, :, 1:10], s3.rearrange("p k a b -> p k (a b)"), 1.0 / 1024.0
    )
    # ---- scale 1 ----
    t1 = pool.tile([P, ko, 1], f, tag="t1")
    nc.vector.reduce_sum(t1, s3.rearrange("p k a b -> p k (a b)"), axis=AX.X)
    nc.vector.tensor_scalar_mul(pf[:, :, 0:1], t1, 1.0 / 9216.0)
    # ---- scale 8 (cells of 3x3 grid entries = 12x12 px) ----
    c8 = pool.tile([P, ko, 24, 8], f, tag="c8")
    nc.vector.reduce_sum(c8, g24.rearrange("p k a (b bi) -> p k a b bi", bi=3), axis=AX.X)
    v8 = c8.rearrange("p k (a ai) b -> p k a ai b", ai=3)
    s8 = pool.tile([P, ko, 8, 8], f, tag="s8")
    nc.vector.tensor_add(s8, v8[:, :, :, 0, :], v8[:, :, :, 1, :])
    nc.vector.tensor_add(s8, s8, v8[:, :, :, 2, :])
    nc.vector.tensor_scalar_mul(
        pf[:, :, 46:110], s8.rearrange("p k a b -> p k (a b)"), 1.0 / 144.0
    )


def _rowpool(nc, gcol, g24, ko):
    """gcol: [P, ko, 2304] col-pooled sums (layout h*24+wb) -> g24 [P,ko,24,24]."""
    v = gcol.rearrange("p k (hb hi wb) -> p k hb hi wb", hi=4, wb=24)
    nc.vector.tensor_add(g24, v[:, :, :, 0, :], v[:, :, :, 1, :])
    nc.vector.tensor_add(g24, g24, v[:, :, :, 2, :])
    nc.vector.tensor_add(g24, g24, v[:, :, :, 3, :])


def _build_body(ctx: ExitStack, tc: tile.TileContext, x_d, wkt_d, wvt_d, wct_d,
                bwt_d, bk_d, bf_d, out_d):
    nc = tc.nc

    consts = ctx.enter_context(tc.tile_pool(name="consts", bufs=1))
    big = ctx.enter_context(tc.tile_pool(name="big", bufs=1))
    stage = ctx.enter_context(tc.tile_pool(name="stage", bufs=2))
    poolb = ctx.enter_context(tc.tile_pool(name="poolb", bufs=1))
    work = ctx.enter_context(tc.tile_pool(name="work", bufs=2))
    outp = ctx.enter_context(tc.tile_pool(name="outp", bufs=3))
    psum = ctx.enter_context(tc.tile_pool(name="psum", bufs=4, space="PSUM"))
    psum2 = ctx.enter_context(tc.tile_pool(name="psum2", bufs=2, space="PSUM"))

    # ---- weights / constants into SBUF ----
    wkt = consts.tile([P, 4, CK], BF16)
    nc.sync.dma_start(wkt, wkt_d[:].rearrange("(kc p) m -> p kc m", p=P))
    wvt = consts.tile([P, 4, CV], BF16)
    nc.sync.dma_start(wvt, wvt_d[:].rearrange("(kc p) m -> p kc m", p=P))
    wct = consts.tile([P, 2, COUT], BF16)
    nc.sync.dma_start(wct, wct_d[:].rearrange("(kc p) m -> p kc m", p=P))
    bwt = consts.tile([P, 4, COUT], BF16)
    nc.sync.dma_start(bwt, bwt_d[:].rearrange("(kc p) m -> p kc m", p=P))
    bkb = consts.tile([P, 2], F32)
    nc.sync.dma_start(bkb, bk_d[:].rearrange("(mc p) -> p mc", p=P))
    bfb = consts.tile([P, 4], F32)
    nc.sync.dma_start(bfb, bf_d[:].rearrange("(mc p) -> p mc", p=P))
    ones_col = consts.tile([S, 1], BF16)
    nc.vector.memset(ones_col, 1.0)
    negs = consts.tile([1, S], BF16)
    nc.vector.memset(negs, -16.0)
    ident = consts.tile([P, P], BF16)
    make_identity(nc, ident)

    # ---- persistent full-res activations (bf16) ----
    kfb = big.tile([P, 2, N], BF16)    # relu(key/query features)
    outb = big.tile([P, 4, N], BF16)   # Bw' @ X partial of the output conv
    gcolx = poolb.tile([P, 4, 2304], BF16)  # X col-pool sums (4-px groups)
    g24k = poolb.tile([P, 2, 24, 24], F32)  # KF 4x4-block sums (24x24 grid)

    xv = x_d[:].rearrange("(kc p) n -> p kc n", p=P)
    ov = out_d[:].rearrange("(mc p) n -> p mc n", p=P)
    kg = kfb.rearrange("p k (h w) -> p k h w", w=W)
    hc_done = 0

    # ---- phase 1: stream X; KF, OUTB, col-pools ----
    for t in range(NTILES):
        xs = stage.tile([P, 4, NT], F32, tag="xs")
        nc.sync.dma_start(xs, xv[:, :, ts(t, NT)])
        xt = stage.tile([P, 4, NT], BF16, tag="xt")
        nc.vector.tensor_copy(out=xt, in_=xs)
        for mc in range(2):
            ps = psum.tile([P, NT], F32, tag="big_ps")
            for kc in range(4):
                nc.tensor.matmul(ps, wkt[:, kc, ts(mc, P)], xt[:, kc, :],
                                 start=(kc == 0), stop=(kc == 3))
            nc.scalar.activation(kfb[:, mc, ts(t, NT)], ps, RELU,
                                 bias=bkb[:, mc:mc + 1])
        for mc in range(4):
            ps = psum.tile([P, NT], F32, tag="big_ps")
            for kc in range(4):
                nc.tensor.matmul(ps, bwt[:, kc, ts(mc, P)], xt[:, kc, :],
                                 start=(kc == 0), stop=(kc == 3))
            nc.scalar.activation(outb[:, mc, ts(t, NT)], ps, COPY)
        # col-pool this tile's 512 columns (128 groups of 4 px)
        with nc.allow_low_precision(reason="pyramid-pool partial sums in bf16"):
            nc.vector.reduce_sum(
                gcolx[:, :, ts(t, P)],
                xt.rearrange("p k (g gi) -> p k g gi", gi=4), axis=AX.X)
        # KF pooling straight to the 24x24 grid, in 12-row chunks (kfb is
        # persistent, so a chunk can span tile boundaries)
        while hc_done < 8 and (hc_done + 1) * 1152 <= (t + 1) * NT:
            hc = hc_done
            for k in range(2):
                src_ap = kg[:, k, ts(hc, 12), :].rearrange(
                    "p (hb hi) (wb wi) -> p hb wb hi wi", hi=4, wi=4)
                nc.vector.reduce_sum(g24k[:, k, ts(hc, 3), :], src_ap, axis=AX.XY)
            hc_done += 1

    # ---- phase 2: row pools, pyramid means, VT ----
    g24x = poolb.tile([P, 4, 24, 24], BF16)
    _rowpool(nc, gcolx, g24x, 4)
    pfx = consts.tile([P, 4, S], BF16)
    _psp_stage2(nc, poolb, g24x, 4, pfx)

    vt_ps = psum2.tile([P, CV], F32, tag="sim_ps")
    for kc in range(4):
        nc.tensor.matmul(vt_ps[:S, :], pfx[:, kc, :], wvt[:, kc, :],
                         start=(kc == 0), stop=(kc == 3))
    vt = consts.tile([S, CV], BF16)
    nc.scalar.copy(vt, vt_ps[:S, :])

    kpx = consts.tile([P, 2, S], BF16)
    _psp_stage2(nc, poolb, g24k, 2, kpx)

    # ---- phase 3: attention + output, streamed over N tiles ----
    for t in range(NTILES):
        sim_ps = psum2.tile([P, NT], F32, tag="sim_ps")
        nc.tensor.matmul(sim_ps[:S, :], kpx[:, 0, :], kfb[:, 0, ts(t, NT)],
                         start=True, stop=False)
        nc.tensor.matmul(sim_ps[:S, :], kpx[:, 1, :], kfb[:, 1, ts(t, NT)],
                         start=False, stop=True)
        e1 = work.tile([P, NT], BF16, tag="e1")
        nc.scalar.activation(e1[:S, :], sim_ps[:S, :], EXP, scale=0.0625)
        cs_ps = psum2.tile([1, NT], F32, tag="aux_ps")
        nc.tensor.matmul(cs_ps, ones_col, e1[:S, :], start=True, stop=True)
        lrow = work.tile([1, NT], BF16, tag="lrow")
        nc.scalar.activation(lrow, cs_ps, LN)
        # rank-1 update: sim += (-16) * ln(colsum), so exp(sim/16) is normalized
        nc.tensor.matmul(sim_ps[:S, :], negs, lrow, start=False, stop=True,
                         skip_group_check=True)
        en = work.tile([P, NT], BF16, tag="en")
        nc.scalar.activation(en[:S, :], sim_ps[:S, :], EXP, scale=0.0625)

        ctxb = work.tile([P, 2, NT], BF16, tag="ctxb")
        for vc in range(2):
            ctx_ps = psum2.tile([P, NT], F32, tag="aux_ps")
            nc.tensor.matmul(ctx_ps, vt[:, ts(vc, P)], en[:S, :],
                             start=True, stop=True)
            nc.vector.tensor_copy(out=ctxb[:, vc, :], in_=ctx_ps)

        for mc in range(4):
            out_ps = psum.tile([P, NT], F32, tag="big_ps")
            nc.tensor.matmul(out_ps, ident, outb[:, mc, ts(t, NT)],
                             start=True, stop=False)
            nc.tensor.matmul(out_ps, wct[:, 0, ts(mc, P)], ctxb[:, 0, :],
                             start=False, stop=False)
            nc.tensor.matmul(out_ps, wct[:, 1, ts(mc, P)], ctxb[:, 1, :],
                             start=False, stop=True)
            osb = outp.tile([P, NT], F32, tag="osb")
            if mc < 2:
                nc.scalar.activation(osb, out_ps, RELU, bias=bfb[:, mc:mc + 1])
            else:
                nc.vector.tensor_scalar(osb, out_ps, scalar1=bfb[:, mc:mc + 1],
                                        scalar2=0.0, op0=ADD, op1=MAX)
            nc.sync.dma_start(ov[:, mc, ts(t, NT)], osb)


def _patch_act_tables():
    """Force every activation onto the one table that holds Exp, Ln, Relu and
    Copy together (`natural_log_exp_and_others`), so the kernel does a single
    ACT_TABLE_LOAD instead of reloading on every Exp<->Ln<->Relu switch.

    Table ids are positional (index into act_info.json), so we keep the dict
    order/size and just empty the other entries.
    """
    import concourse.hw_specs as hw_specs

    if getattr(bacc, "_apnb_act_patch", False):
        return
    orig = hw_specs.get_activation_tables

    def patched(module_arch):
        tabs = orig(module_arch)
        keep = "natural_log_exp_and_others"
        if keep not in tabs:
            return tabs
        return {k: (v if k == keep else set()) for k, v in tabs.items()}

    bacc.get_activation_tables = patched
    bacc._apnb_act_patch = True


def build_nc():
    _patch_act_tables()
    nc = bacc.Bacc("TRN2", target_bir_lowering=False, debug=False)
    x_d = nc.declare_dram_parameter("x", [CIN, N], F32, isOutput=False)
    wkt_d = nc.declare_dram_parameter("wkt", [CIN, CK], BF16, isOutput=False)
    wvt_d = nc.declare_dram_parameter("wvt", [CIN, CV], BF16, isOutput=False)
    wct_d = nc.declare_dram_parameter("wct", [CV, COUT], BF16, isOutput=False)
    bwt_d = nc.declare_dram_parameter("bwt", [CIN, COUT], BF16, isOutput=False)
    bk_d = nc.declare_dram_parameter("bk", [CK], F32, isOutput=False)
    bf_d = nc.declare_dram_parameter("bf", [COUT], F32, isOutput=False)
    out_d = nc.declare_dram_parameter("out", [COUT, N], F32, isOutput=True)
    with tile.TileContext(nc) as tc:
        with ExitStack() as ctx:
            _build_body(ctx, tc, x_d, wkt_d, wvt_d, wct_d, bwt_d, bk_d, bf_d,
                        out_d)
    nc.compile()
    return nc


def _get_nc():
    global _NC
    if _NC is None:
        _NC = build_nc()
    return _NC


def fold_params(Wk, bk, gk, betak, mk, vk, Wv, bv, Ww, bw, Wo, bo, go, betao,
                mo, vo):
    """Fold BN params + the Ww conv into effective weights (all f32 numpy)."""
    bf16 = ml_dtypes.bfloat16
    sk = gk / np.sqrt(vk + EPS)
    Wk_f = sk[:, None] * Wk
    bk_f = (bk - mk) * sk + betak
    so = go / np.sqrt(vo + EPS)
    A = so[:, None] * Wo[:, :CIN]      # applies to ctx2 = Ww@ctx + bw
    Bw = so[:, None] * Wo[:, CIN:]     # applies to feats
    b0 = (bo - mo) * so + betao
    Wc = A @ Ww                        # (COUT, CV)
    # attn rows sum to 1  =>  value bias bv contributes Wc @ bv everywhere
    bf_ = b0 + A @ bw + Wc @ bv
    return {
        "wkt": np.ascontiguousarray(Wk_f.T).astype(bf16),
        "wvt": np.ascontiguousarray(Wv.T).astype(bf16),
        "wct": np.ascontiguousarray(Wc.T).astype(bf16),
        "bwt": np.ascontiguousarray(Bw.T).astype(bf16),
        "bk": bk_f.astype(np.float32),
        "bf": bf_.astype(np.float32),
    }


def kernel(**inputs):
    global LAST_RESULT
    feats = np.asarray(inputs["feats"], np.float32)
    B = feats.shape[0]
    assert feats.shape == (B, CIN, H, W) and B == N_CORES

    common = fold_params(
        np.asarray(inputs["Wk"], np.float32), np.asarray(inputs["bk"], np.float32),
        np.asarray(inputs["gk"], np.float32), np.asarray(inputs["betak"], np.float32),
        np.asarray(inputs["mk"], np.float32), np.asarray(inputs["vk"], np.float32),
        np.asarray(inputs["Wv"], np.float32), np.asarray(inputs["bv"], np.float32),
        np.asarray(inputs["Ww"], np.float32), np.asarray(inputs["bw"], np.float32),
        np.asarray(inputs["Wo"], np.float32), np.asarray(inputs["bo"], np.float32),
        np.asarray(inputs["go"], np.float32), np.asarray(inputs["betao"], np.float32),
        np.asarray(inputs["mo"], np.float32), np.asarray(inputs["vo"], np.float32),
    )
    in_maps = [
        {"x": np.ascontiguousarray(feats[i].reshape(CIN, N)), **common}
        for i in range(N_CORES)
    ]
    nc = _get_nc()
    res = run_bass_kernel_spmd(nc, in_maps, core_ids=list(range(N_CORES)),
                               trace=PROFILE)
    LAST_RESULT = res
    out = np.stack([res.results[i]["out"].reshape(COUT, H, W)
                    for i in range(N_CORES)])
    return out.astype(np.float32)


# revision 11
# speedup vs baseline: 1.5493x; 1.0261x over previous
"""APNB (asymmetric pyramid non-local block) sparse-attention kernel for 8 TRN2 NeuronCores.

Strategy: pure data-parallel over batch (B=8 -> one batch element per core, no
collectives). Per core, the whole block is computed with bf16 TensorE GEMMs
(f32 PSUM accumulation):

  host:        BN+bias folded into conv weights; W (value->out conv) folded
               into the first half of the output conv: Wc = (so*Wo1) @ Ww.
               X pre-cast to bf16 (halves input DMA, no on-chip cast).
  phase 1 (streamed over blocks of 4 N-tiles, X transient in SBUF):
               KF   = relu(Wk' @ X + bk')          (256, 9216)  persistent bf16
               OUTB = Bw' @ X                      (512, 9216)  persistent bf16
               col-pool partial sums of X and KF   (DVE reduce)
               Matmuls are ordered stationary-major (one LDWEIGHTS per 4 MMs)
               with adjacent MMs hitting different PSUM banks so fill/drain
               pipeline at the ~N/2.4GHz streaming rate.
  phase 2:     row-pool + PSP pyramid (1,3,6,8) means -> PF, KP
               VT = PF^T @ Wv^T                    (110, 256)
  phase 3 (streamed over pairs of N-tiles):
               SIM^T = KP^T @ KF                   (110, N)
               attn  = exp(s*SIM - ln(colsum))     (div-free softmax; the ln is
                       folded into PSUM via a rank-1 matmul)
               CTX^T = VT^T @ attn                 (256, N)
               OUT   = relu(OUTB + Wc @ CTX^T + b')  (OUTB re-injected into
                       PSUM with an identity matmul)

The softmax needs no max-subtraction: |s*sim| is O(1) for this problem's data
distribution, so exp is safe in f32.
"""

import numpy as np
import ml_dtypes
from contextlib import ExitStack

import concourse.bass as bass
import concourse.bacc as bacc
import concourse.mybir as mybir
import concourse.tile as tile
from concourse.bass import ts, ds
from concourse.bass_utils import run_bass_kernel_spmd
from concourse.masks import make_identity

P = 128
CIN, CK, CV, COUT = 512, 256, 256, 512
H = W = 96
N = H * W              # 9216
NT = 512               # matmul free-dim tile
NTILES = N // NT       # 18
S = 110                # pooled tokens: 1+9+36+64
EPS = 1e-5
F32 = mybir.dt.float32
BF16 = mybir.dt.bfloat16
RELU = mybir.ActivationFunctionType.Relu
EXP = mybir.ActivationFunctionType.Exp
LN = mybir.ActivationFunctionType.Ln
COPY = mybir.ActivationFunctionType.Copy
ADD = mybir.AluOpType.add
MAX = mybir.AluOpType.max
AX = mybir.AxisListType

N_CORES = 8

# set by test harness to capture profile info
PROFILE = False
LAST_RESULT = None

_NC = None


def _psp_stage2(nc, pool, g24, ko, pf):
    """g24: [P, ko, 24, 24] 4x4-pixel SUMS. pf: [P, ko, 110] bf16 pyramid means.

    Reference concat order (1, 3, 6, 8). Scale-6 cells are 4x4 grid cells;
    scale-8 are 3x3; scale-3 = 2x2 of scale-6; scale-1 = sum of all scale-3.
    """
    f = F32
    # ---- scale 6 (cells of 4x4 grid entries = 16x16 px) ----
    c6 = pool.tile([P, ko, 24, 6], BF16, tag="c6")
    with nc.allow_low_precision(reason="pool partials"):
        nc.vector.reduce_sum(c6, g24.rearrange("p k a (b bi) -> p k a b bi", bi=4), axis=AX.X)
    v6 = c6.rearrange("p k (a ai) b -> p k a ai b", ai=4)
    s6 = pool.tile([P, ko, 6, 6], f, tag="s6")
    nc.vector.tensor_add(s6, v6[:, :, :, 0, :], v6[:, :, :, 1, :])
    nc.vector.tensor_add(s6, s6, v6[:, :, :, 2, :])
    nc.vector.tensor_add(s6, s6, v6[:, :, :, 3, :])
    nc.vector.tensor_scalar_mul(
        pf[:, :, 10:46], s6.rearrange("p k a b -> p k (a b)"), 1.0 / 256.0
    )
    # ---- scale 3 (2x2 of scale-6 cells = 32x32 px) ----
    c3 = pool.tile([P, ko, 6, 3], f, tag="c3")
    nc.vector.reduce_sum(c3, s6.rearrange("p k a (b bi) -> p k a b bi", bi=2), axis=AX.X)
    v3 = c3.rearrange("p k (a ai) b -> p k a ai b", ai=2)
    s3 = pool.tile([P, ko, 3, 3], f, tag="s3")
    nc.vector.tensor_add(s3, v3[:, :, :, 0, :], v3[:, :, :, 1, :])
    nc.vector.tensor_scalar_mul(
        pf[:, :, 1:10], s3.rearrange("p k a b -> p k (a b)"), 1.0 / 1024.0
    )
    # ---- scale 1 ----
    t1 = pool.tile([P, ko, 1], f, tag="t1")
    nc.vector.reduce_sum(t1, s3.rearrange("p k a b -> p k (a b)"), axis=AX.X)
    nc.vector.tensor_scalar_mul(pf[:, :, 0:1], t1, 1.0 / 9216.0)
    # ---- scale 8 (cells of 3x3 grid entries = 12x12 px) ----
    c8 = pool.tile([P, ko, 24, 8], BF16, tag="c8")
    with nc.allow_low_precision(reason="pool partials"):
        nc.vector.reduce_sum(c8, g24.rearrange("p k a (b bi) -> p k a b bi", bi=3), axis=AX.X)
    v8 = c8.rearrange("p k (a ai) b -> p k a ai b", ai=3)
    s8 = pool.tile([P, ko, 8, 8], f, tag="s8")
    nc.vector.tensor_add(s8, v8[:, :, :, 0, :], v8[:, :, :, 1, :])
    nc.vector.tensor_add(s8, s8, v8[:, :, :, 2, :])
    nc.vector.tensor_scalar_mul(
        pf[:, :, 46:110], s8.rearrange("p k a b -> p k (a b)"), 1.0 / 144.0
    )


def _build_body(ctx: ExitStack, tc: tile.TileContext, x_d, wkt_d, wvt_d, wct_d,
                bwt_d, bk_d, bf_d, out_d):
    nc = tc.nc

    consts = ctx.enter_context(tc.tile_pool(name="consts", bufs=1))
    big = ctx.enter_context(tc.tile_pool(name="big", bufs=1))
    stage = ctx.enter_context(tc.tile_pool(name="stage", bufs=2))
    poolb = ctx.enter_context(tc.tile_pool(name="poolb", bufs=1))
    work = ctx.enter_context(tc.tile_pool(name="work", bufs=2))
    outp = ctx.enter_context(tc.tile_pool(name="outp", bufs=2))

    # ---- weights / constants into SBUF ----
    wkt = consts.tile([P, 4, CK], BF16)
    nc.sync.dma_start(wkt, wkt_d[:].rearrange("(kc p) m -> p kc m", p=P))
    wvt = consts.tile([P, 4, CV], BF16)
    nc.sync.dma_start(wvt, wvt_d[:].rearrange("(kc p) m -> p kc m", p=P))
    wct = consts.tile([P, 2, COUT], BF16)
    nc.sync.dma_start(wct, wct_d[:].rearrange("(kc p) m -> p kc m", p=P))
    bwt = consts.tile([P, 4, COUT], BF16)
    nc.sync.dma_start(bwt, bwt_d[:].rearrange("(kc p) m -> p kc m", p=P))
    bkb = consts.tile([P, 2], F32)
    nc.sync.dma_start(bkb, bk_d[:].rearrange("(mc p) -> p mc", p=P))
    bfb = consts.tile([P, 4], F32)
    nc.sync.dma_start(bfb, bf_d[:].rearrange("(mc p) -> p mc", p=P))
    ones_col = consts.tile([S, 1], BF16)
    nc.vector.memset(ones_col, 1.0)
    negs = consts.tile([1, S], BF16)
    nc.vector.memset(negs, -16.0)
    ident = consts.tile([P, P], BF16)
    make_identity(nc, ident)

    # ---- persistent full-res activations (bf16) ----
    kfb = big.tile([P, 2, N], BF16)    # relu(key/query features)
    outb = big.tile([P, 4, N], BF16)   # Bw' @ X partial of the output conv
    gcolx = poolb.tile([P, 4, 2304], BF16)  # X col-pool sums (4-px groups)
    g24k = poolb.tile([P, 2, 24, 24], F32)  # KF 4x4-block sums (24x24 grid)

    xv = x_d[:].rearrange("(kc p) n -> p kc n", p=P)
    ov = out_d[:].rearrange("(mc p) n -> p mc n", p=P)
    kg = kfb.rearrange("p k (h w) -> p k h w", w=W)
    hc_done = 0

    # ---- phase 1: stream X in blocks of 4 tiles; KF, OUTB, col-pools ----
    # Loop order inside a block: stationary-major (mc, kc) outer, block-tile j
    # inner, so one LDWEIGHTS serves 4 matmuls and adjacent matmuls write
    # different PSUM banks (full fill/drain pipelining).
    blocks = [(b * 4, min(4, NTILES - b * 4)) for b in range((NTILES + 3) // 4)]
    with tc.tile_pool(name="psA", bufs=2, space="PSUM") as psA:
        for b0, jb in blocks:
            c0, cw = b0 * NT, jb * NT
            xt_full = stage.tile([P, 4, 4 * NT], BF16, tag="xt")
            xt = xt_full[:, :, :cw]
            nc.sync.dma_start(xt, xv[:, :, ds(c0, cw)])
            # KF: mc 0..1 from wkt
            for mc in range(2):
                ps_full = psA.tile([P, 4, NT], F32, tag="mm4")
                ps = ps_full[:, :jb, :]
                for kc in range(4):
                    for j in range(jb):
                        nc.tensor.matmul(ps[:, j, :], wkt[:, kc, ts(mc, P)],
                                         xt[:, kc, ts(j, NT)],
                                         start=(kc == 0), stop=(kc == 3))
                nc.scalar.activation(kfb[:, mc, ds(c0, cw)],
                                     ps.rearrange("p j n -> p (j n)"), RELU,
                                     bias=bkb[:, mc:mc + 1])
            # OUTB: mc 0..3 from bwt
            for mc in range(4):
                ps_full = psA.tile([P, 4, NT], F32, tag="mm4")
                ps = ps_full[:, :jb, :]
                for kc in range(4):
                    for j in range(jb):
                        nc.tensor.matmul(ps[:, j, :], bwt[:, kc, ts(mc, P)],
                                         xt[:, kc, ts(j, NT)],
                                         start=(kc == 0), stop=(kc == 3))
                nc.scalar.activation(outb[:, mc, ds(c0, cw)],
                                     ps.rearrange("p j n -> p (j n)"), COPY)
            # X col-pool for this block (4-px groups along w)
            with nc.allow_low_precision(reason="pyramid-pool partials in bf16"):
                nc.vector.reduce_sum(
                    gcolx[:, :, ds(b0 * P, jb * P)],
                    xt.rearrange("p k (g gi) -> p k g gi", gi=4), axis=AX.X)
            # KF pooling straight to the 24x24 grid, in 12-row chunks (kfb is
            # persistent, so chunks can span tile boundaries)
            while hc_done < 8 and (hc_done + 1) * 1152 <= c0 + cw:
                hc = hc_done
                for k in range(2):
                    src_ap = kg[:, k, ts(hc, 12), :].rearrange(
                        "p (hb hi) (wb wi) -> p hb wb hi wi", hi=4, wi=4)
                    nc.vector.reduce_sum(g24k[:, k, ts(hc, 3), :], src_ap,
                                         axis=AX.XY)
                hc_done += 1

    with tc.tile_pool(name="psB", bufs=2, space="PSUM") as psB, \
         tc.tile_pool(name="psO", bufs=2, space="PSUM") as psO:
        # ---- phase 2: row pools, pyramid means, VT ----
        g24x = poolb.tile([P, 4, 24, 24], BF16)
        v = gcolx.rearrange("p k (hb hi wb) -> p k hb hi wb", hi=4, wb=24)
        nc.vector.tensor_add(g24x, v[:, :, :, 0, :], v[:, :, :, 1, :])
        nc.vector.tensor_add(g24x, g24x, v[:, :, :, 2, :])
        nc.vector.tensor_add(g24x, g24x, v[:, :, :, 3, :])
        pfx = consts.tile([P, 4, S], BF16)
        _psp_stage2(nc, poolb, g24x, 4, pfx)

        vt_ps = psB.tile([P, NT], F32, tag="sim")
        for kc in range(4):
            nc.tensor.matmul(vt_ps[:S, :CV], pfx[:, kc, :], wvt[:, kc, :],
                             start=(kc == 0), stop=(kc == 3))
        vt = consts.tile([S, CV], BF16)
        nc.scalar.copy(vt, vt_ps[:S, :CV])

        kpx = consts.tile([P, 2, S], BF16)
        _psp_stage2(nc, poolb, g24k, 2, kpx)

        # ---- phase 3: attention + output, streamed over pairs of N tiles ----
        for tp in range(NTILES // 2):
            tt = (2 * tp, 2 * tp + 1)
            ens = []
            ctxb = work.tile([P, 2, 2, NT], BF16, tag="ctxb")  # [vc, tloc]
            for tloc, t in enumerate(tt):
                sim_ps = psB.tile([P, NT], F32, tag="sim")
                nc.tensor.matmul(sim_ps[:S, :], kpx[:, 0, :],
                                 kfb[:, 0, ts(t, NT)], start=True, stop=False)
                nc.tensor.matmul(sim_ps[:S, :], kpx[:, 1, :],
                                 kfb[:, 1, ts(t, NT)], start=False, stop=True)
                e1 = work.tile([P, NT], BF16, tag="e1")
                nc.scalar.activation(e1[:S, :], sim_ps[:S, :], EXP, scale=0.0625)
                cs_ps = psB.tile([1, NT], F32, tag="aux")
                nc.tensor.matmul(cs_ps, ones_col, e1[:S, :], start=True, stop=True)
                lrow = work.tile([1, NT], BF16, tag="lrow")
                nc.scalar.activation(lrow, cs_ps, LN)
                # rank-1: sim += (-16)*ln(colsum) => exp(sim/16) is normalized
                nc.tensor.matmul(sim_ps[:S, :], negs, lrow, start=False,
                                 stop=True, skip_group_check=True)
                en = work.tile([P, NT], BF16, tag="en")
                nc.scalar.activation(en[:S, :], sim_ps[:S, :], EXP, scale=0.0625)
                ens.append(en)
            for vc in range(2):
                ctx_ps = psB.tile([P, NT], F32, tag="aux")
                for tloc in range(2):
                    if tloc == 1:
                        ctx_ps = psB.tile([P, NT], F32, tag="aux")
                    nc.tensor.matmul(ctx_ps, vt[:, ts(vc, P)],
                                     ens[tloc][:S, :], start=True, stop=True)
                    nc.vector.tensor_copy(out=ctxb[:, vc, tloc, :], in_=ctx_ps)
            for mc in range(4):
                ops = psO.tile([P, 2, NT], F32, tag="outp")
                nc.tensor.matmul(ops[:, 0, :], ident, outb[:, mc, ts(tt[0], NT)],
                                 start=True, stop=False)
                nc.tensor.matmul(ops[:, 1, :], ident, outb[:, mc, ts(tt[1], NT)],
                                 start=True, stop=False)
                for vc in range(2):
                    last = vc == 1
                    nc.tensor.matmul(ops[:, 0, :], wct[:, vc, ts(mc, P)],
                                     ctxb[:, vc, 0, :], start=False, stop=last)
                    nc.tensor.matmul(ops[:, 1, :], wct[:, vc, ts(mc, P)],
                                     ctxb[:, vc, 1, :], start=False, stop=last)
                osb = outp.tile([P, 2, NT], F32, tag="osb")
                opsv = ops.rearrange("p j n -> p (j n)")
                osv = osb.rearrange("p j n -> p (j n)")
                if mc < 3:
                    nc.scalar.activation(osv, opsv, RELU, bias=bfb[:, mc:mc + 1])
                else:
                    nc.vector.tensor_scalar(osv, opsv, scalar1=bfb[:, mc:mc + 1],
                                            scalar2=0.0, op0=ADD, op1=MAX)
                nc.sync.dma_start(ov[:, mc, ds(2 * tp * NT, 2 * NT)], osv)


def _patch_act_tables():
    """Force every activation onto the one table that holds Exp, Ln, Relu and
    Copy together (`natural_log_exp_and_others`), so the kernel does a single
    ACT_TABLE_LOAD instead of reloading on every Exp<->Ln<->Relu switch.

    Table ids are positional (index into act_info.json), so we keep the dict
    order/size and just empty the other entries.
    """
    import concourse.hw_specs as hw_specs

    if getattr(bacc, "_apnb_act_patch", False):
        return
    orig = hw_specs.get_activation_tables

    def patched(module_arch):
        tabs = orig(module_arch)
        keep = "natural_log_exp_and_others"
        if keep not in tabs:
            return tabs
        return {k: (v if k == keep else set()) for k, v in tabs.items()}

    bacc.get_activation_tables = patched
    bacc._apnb_act_patch = True


def build_nc():
    _patch_act_tables()
    nc = bacc.Bacc("TRN2", target_bir_lowering=False, debug=False)
    x_d = nc.declare_dram_parameter("x", [CIN, N], BF16, isOutput=False)
    wkt_d = nc.declare_dram_parameter("wkt", [CIN, CK], BF16, isOutput=False)
    wvt_d = nc.declare_dram_parameter("wvt", [CIN, CV], BF16, isOutput=False)
    wct_d = nc.declare_dram_parameter("wct", [CV, COUT], BF16, isOutput=False)
    bwt_d = nc.declare_dram_parameter("bwt", [CIN, COUT], BF16, isOutput=False)
    bk_d = nc.declare_dram_parameter("bk", [CK], F32, isOutput=False)
    bf_d = nc.declare_dram_parameter("bf", [COUT], F32, isOutput=False)
    out_d = nc.declare_dram_parameter("out", [COUT, N], F32, isOutput=True)
    with tile.TileContext(nc) as tc:
        with ExitStack() as ctx:
            _build_body(ctx, tc, x_d, wkt_d, wvt_d, wct_d, bwt_d, bk_d, bf_d,
                        out_d)
    nc.compile()
    return nc


def _get_nc():
    global _NC
    if _NC is None:
        _NC = build_nc()
    return _NC


def fold_params(Wk, bk, gk, betak, mk, vk, Wv, bv, Ww, bw, Wo, bo, go, betao,
                mo, vo):
    """Fold BN params + the Ww conv into effective weights (all f32 numpy)."""
    bf16 = ml_dtypes.bfloat16
    sk = gk / np.sqrt(vk + EPS)
    Wk_f = sk[:, None] * Wk
    bk_f = (bk - mk) * sk + betak
    so = go / np.sqrt(vo + EPS)
    A = so[:, None] * Wo[:, :CIN]      # applies to ctx2 = Ww@ctx + bw
    Bw = so[:, None] * Wo[:, CIN:]     # applies to feats
    b0 = (bo - mo) * so + betao
    Wc = A @ Ww                        # (COUT, CV)
    # attn rows sum to 1  =>  value bias bv contributes Wc @ bv everywhere
    bf_ = b0 + A @ bw + Wc @ bv
    return {
        "wkt": np.ascontiguousarray(Wk_f.T).astype(bf16),
        "wvt": np.ascontiguousarray(Wv.T).astype(bf16),
        "wct": np.ascontiguousarray(Wc.T).astype(bf16),
        "bwt": np.ascontiguousarray(Bw.T).astype(bf16),
        "bk": bk_f.astype(np.float32),
        "bf": bf_.astype(np.float32),
    }


def kernel(**inputs):
    global LAST_RESULT
    feats = np.asarray(inputs["feats"], np.float32)
    B = feats.shape[0]
    assert feats.shape == (B, CIN, H, W) and B == N_CORES

    common = fold_params(
        np.asarray(inputs["Wk"], np.float32), np.asarray(inputs["bk"], np.float32),
        np.asarray(inputs["gk"], np.float32), np.asarray(inputs["betak"], np.float32),
        np.asarray(inputs["mk"], np.float32), np.asarray(inputs["vk"], np.float32),
        np.asarray(inputs["Wv"], np.float32), np.asarray(inputs["bv"], np.float32),
        np.asarray(inputs["Ww"], np.float32), np.asarray(inputs["bw"], np.float32),
        np.asarray(inputs["Wo"], np.float32), np.asarray(inputs["bo"], np.float32),
        np.asarray(inputs["go"], np.float32), np.asarray(inputs["betao"], np.float32),
        np.asarray(inputs["mo"], np.float32), np.asarray(inputs["vo"], np.float32),
    )
    bf16 = ml_dtypes.bfloat16
    in_maps = [
        {"x": np.ascontiguousarray(feats[i].reshape(CIN, N)).astype(bf16),
         **common}
        for i in range(N_CORES)
    ]
    nc = _get_nc()
    res = run_bass_kernel_spmd(nc, in_maps, core_ids=list(range(N_CORES)),
                               trace=PROFILE)
    LAST_RESULT = res
    out = np.stack([res.results[i]["out"].reshape(COUT, H, W)
                    for i in range(N_CORES)])
    return out.astype(np.float32)


# revision 15
# speedup vs baseline: 1.8030x; 1.1637x over previous
"""APNB (asymmetric pyramid non-local block) sparse-attention kernel for 8 TRN2 NeuronCores.

Strategy: pure data-parallel over batch (B=8 -> one batch element per core, no
collectives). Per core, the whole block is computed with bf16 TensorE GEMMs
(f32 PSUM accumulation):

  host:        BN+bias folded into conv weights; W (value->out conv) folded
               into the first half of the output conv: Wc = (so*Wo1) @ Ww.
               X pre-cast to bf16 (halves input DMA, no on-chip cast).
  phase 1 (streamed over blocks of 4 N-tiles, X transient in SBUF):
               KF   = relu(Wk' @ X + bk')          (256, 9216)  persistent bf16
               OUTB = Bw' @ X                      (512, 9216)  persistent bf16
               col-pool partial sums of X and KF   (DVE reduce)
               Matmuls are ordered stationary-major (one LDWEIGHTS per 4 MMs)
               with adjacent MMs hitting different PSUM banks so fill/drain
               pipeline at the ~N/2.4GHz streaming rate.
  phase 2:     row-pool + PSP pyramid (1,3,6,8) means -> PF, KP
               VT = PF^T @ Wv^T                    (110, 256)
  phase 3 (streamed over pairs of N-tiles):
               SIM^T = KP^T @ KF                   (110, N)
               attn  = exp(s*SIM - ln(colsum))     (div-free softmax; the ln is
                       folded into PSUM via a rank-1 matmul)
               CTX^T = VT^T @ attn                 (256, N)
               OUT   = relu(OUTB + Wc @ CTX^T + b')  (OUTB re-injected into
                       PSUM with an identity matmul)

The softmax needs no max-subtraction: |s*sim| is O(1) for this problem's data
distribution, so exp is safe in f32.
"""

import numpy as np
import ml_dtypes
from contextlib import ExitStack

import concourse.bass as bass
import concourse.bacc as bacc
import concourse.mybir as mybir
import concourse.tile as tile
from concourse.bass import ts, ds
from concourse.bass_utils import run_bass_kernel_spmd
from concourse.masks import make_identity

P = 128
CIN, CK, CV, COUT = 512, 256, 256, 512
H = W = 96
N = H * W              # 9216
NT = 512               # matmul free-dim tile
NTILES = N // NT       # 18
S = 110                # pooled tokens: 1+9+36+64
EPS = 1e-5
F32 = mybir.dt.float32
BF16 = mybir.dt.bfloat16
RELU = mybir.ActivationFunctionType.Relu
EXP = mybir.ActivationFunctionType.Exp
LN = mybir.ActivationFunctionType.Ln
COPY = mybir.ActivationFunctionType.Copy
ADD = mybir.AluOpType.add
MAX = mybir.AluOpType.max
AX = mybir.AxisListType

N_CORES = 8

# set by test harness to capture profile info
PROFILE = False
LAST_RESULT = None

_NC = None


def _psp_stage2(nc, pool, g24, ko, pf):
    """g24: [P, ko, 24, 24] 4x4-pixel SUMS. pf: [P, ko, 110] bf16 pyramid means.

    Reference concat order (1, 3, 6, 8). Scale-6 cells are 4x4 grid cells;
    scale-8 are 3x3; scale-3 = 2x2 of scale-6; scale-1 = sum of all scale-3.
    """
    f = F32
    # ---- scale 6 (cells of 4x4 grid entries = 16x16 px) ----
    c6 = pool.tile([P, ko, 24, 6], BF16, tag="c6")
    with nc.allow_low_precision(reason="pool partials"):
        nc.vector.reduce_sum(c6, g24.rearrange("p k a (b bi) -> p k a b bi", bi=4), axis=AX.X)
    v6 = c6.rearrange("p k (a ai) b -> p k a ai b", ai=4)
    s6 = pool.tile([P, ko, 6, 6], f, tag="s6")
    nc.vector.tensor_add(s6, v6[:, :, :, 0, :], v6[:, :, :, 1, :])
    nc.vector.tensor_add(s6, s6, v6[:, :, :, 2, :])
    nc.vector.tensor_add(s6, s6, v6[:, :, :, 3, :])
    nc.vector.tensor_scalar_mul(
        pf[:, :, 10:46], s6.rearrange("p k a b -> p k (a b)"), 1.0 / 256.0
    )
    # ---- scale 3 (2x2 of scale-6 cells = 32x32 px) ----
    c3 = pool.tile([P, ko, 6, 3], f, tag="c3")
    nc.vector.reduce_sum(c3, s6.rearrange("p k a (b bi) -> p k a b bi", bi=2), axis=AX.X)
    v3 = c3.rearrange("p k (a ai) b -> p k a ai b", ai=2)
    s3 = pool.tile([P, ko, 3, 3], f, tag="s3")
    nc.vector.tensor_add(s3, v3[:, :, :, 0, :], v3[:, :, :, 1, :])
    nc.vector.tensor_scalar_mul(
        pf[:, :, 1:10], s3.rearrange("p k a b -> p k (a b)"), 1.0 / 1024.0
    )
    # ---- scale 1 ----
    t1 = pool.tile([P, ko, 1], f, tag="t1")
    nc.vector.reduce_sum(t1, s3.rearrange("p k a b -> p k (a b)"), axis=AX.X)
    nc.vector.tensor_scalar_mul(pf[:, :, 0:1], t1, 1.0 / 9216.0)
    # ---- scale 8 (cells of 3x3 grid entries = 12x12 px) ----
    c8 = pool.tile([P, ko, 24, 8], BF16, tag="c8")
    with nc.allow_low_precision(reason="pool partials"):
        nc.vector.reduce_sum(c8, g24.rearrange("p k a (b bi) -> p k a b bi", bi=3), axis=AX.X)
    v8 = c8.rearrange("p k (a ai) b -> p k a ai b", ai=3)
    s8 = pool.tile([P, ko, 8, 8], f, tag="s8")
    nc.vector.tensor_add(s8, v8[:, :, :, 0, :], v8[:, :, :, 1, :])
    nc.vector.tensor_add(s8, s8, v8[:, :, :, 2, :])
    nc.vector.tensor_scalar_mul(
        pf[:, :, 46:110], s8.rearrange("p k a b -> p k (a b)"), 1.0 / 144.0
    )


def _build_body(ctx: ExitStack, tc: tile.TileContext, x_d, wkt_d, wvt_d, wct_d,
                bwt_d, bk_d, bf_d, out_d):
    nc = tc.nc

    consts = ctx.enter_context(tc.tile_pool(name="consts", bufs=1))
    big = ctx.enter_context(tc.tile_pool(name="big", bufs=1))
    stage = ctx.enter_context(tc.tile_pool(name="stage", bufs=2))
    poolb = ctx.enter_context(tc.tile_pool(name="poolb", bufs=1))
    work = ctx.enter_context(tc.tile_pool(name="work", bufs=2))
    outp = ctx.enter_context(tc.tile_pool(name="outp", bufs=2))

    # ---- weights / constants into SBUF ----
    wkt = consts.tile([P, 4, CK], BF16)
    nc.sync.dma_start(wkt, wkt_d[:].rearrange("(kc p) m -> p kc m", p=P))
    wvt = consts.tile([P, 4, CV], BF16)
    nc.sync.dma_start(wvt, wvt_d[:].rearrange("(kc p) m -> p kc m", p=P))
    wct = consts.tile([P, 2, COUT], BF16)
    nc.sync.dma_start(wct, wct_d[:].rearrange("(kc p) m -> p kc m", p=P))
    bwt = consts.tile([P, 4, COUT], BF16)
    nc.sync.dma_start(bwt, bwt_d[:].rearrange("(kc p) m -> p kc m", p=P))
    bkb = consts.tile([P, 2], F32)
    nc.sync.dma_start(bkb, bk_d[:].rearrange("(mc p) -> p mc", p=P))
    bfb = consts.tile([P, 4], F32)
    nc.sync.dma_start(bfb, bf_d[:].rearrange("(mc p) -> p mc", p=P))
    ones_col = consts.tile([S, 1], BF16)
    nc.vector.memset(ones_col, 1.0)
    negs = consts.tile([1, S], BF16)
    nc.vector.memset(negs, -16.0)
    ident = consts.tile([P, P], BF16)
    make_identity(nc, ident)

    # ---- persistent full-res activations (bf16) ----
    kfb = big.tile([P, 2, N], BF16)    # relu(key/query features)
    outb = big.tile([P, 4, N], BF16)   # Bw' @ X partial of the output conv
    gcolx = poolb.tile([P, 4, 2304], BF16)  # X col-pool sums (4-px groups)
    g24k = poolb.tile([P, 2, 24, 24], F32)  # KF 4x4-block sums (24x24 grid)

    xv = x_d[:].rearrange("(kc p) n -> p kc n", p=P)
    ov = out_d[:].rearrange("(mc p) n -> p mc n", p=P)
    kg = kfb.rearrange("p k (h w) -> p k h w", w=W)
    hc_done = 0

    # ---- phase 1: stream X in blocks of 4 tiles; KF, OUTB, col-pools ----
    # Loop order inside a block: stationary-major (mc, kc) outer, block-tile j
    # inner, so one LDWEIGHTS serves 4 matmuls and adjacent matmuls write
    # different PSUM banks (full fill/drain pipelining).
    blocks = [(b * 4, min(4, NTILES - b * 4)) for b in range((NTILES + 3) // 4)]
    with tc.tile_pool(name="psA", bufs=2, space="PSUM") as psA:
        for b0, jb in blocks:
            c0, cw = b0 * NT, jb * NT
            xt_full = stage.tile([P, 4, 4 * NT], BF16, tag="xt")
            xt = xt_full[:, :, :cw]
            nc.sync.dma_start(xt, xv[:, :, ds(c0, cw)])
            # KF: mc 0..1 from wkt
            for mc in range(2):
                ps_full = psA.tile([P, 4, NT], F32, tag="mm4")
                ps = ps_full[:, :jb, :]
                for kc in range(4):
                    for j in range(jb):
                        nc.tensor.matmul(ps[:, j, :], wkt[:, kc, ts(mc, P)],
                                         xt[:, kc, ts(j, NT)],
                                         start=(kc == 0), stop=(kc == 3))
                nc.scalar.activation(kfb[:, mc, ds(c0, cw)],
                                     ps.rearrange("p j n -> p (j n)"), RELU,
                                     bias=bkb[:, mc:mc + 1])
            # OUTB: mc 0..3 from bwt
            for mc in range(4):
                ps_full = psA.tile([P, 4, NT], F32, tag="mm4")
                ps = ps_full[:, :jb, :]
                for kc in range(4):
                    for j in range(jb):
                        nc.tensor.matmul(ps[:, j, :], bwt[:, kc, ts(mc, P)],
                                         xt[:, kc, ts(j, NT)],
                                         start=(kc == 0), stop=(kc == 3))
                nc.scalar.activation(outb[:, mc, ds(c0, cw)],
                                     ps.rearrange("p j n -> p (j n)"), COPY)
            # X col-pool for this block (4-px groups along w)
            with nc.allow_low_precision(reason="pyramid-pool partials in bf16"):
                nc.vector.reduce_sum(
                    gcolx[:, :, ds(b0 * P, jb * P)],
                    xt.rearrange("p k (g gi) -> p k g gi", gi=4), axis=AX.X)
            # KF pooling straight to the 24x24 grid, in 12-row chunks (kfb is
            # persistent, so chunks can span tile boundaries)
            while hc_done < 8 and (hc_done + 1) * 1152 <= c0 + cw:
                hc = hc_done
                for k in range(2):
                    src_ap = kg[:, k, ts(hc, 12), :].rearrange(
                        "p (hb hi) (wb wi) -> p hb wb hi wi", hi=4, wi=4)
                    nc.vector.reduce_sum(g24k[:, k, ts(hc, 3), :], src_ap,
                                         axis=AX.XY)
                hc_done += 1

    with tc.tile_pool(name="psB", bufs=2, space="PSUM") as psB, \
         tc.tile_pool(name="psO", bufs=2, space="PSUM") as psO:
        # ---- phase 2: row pools, pyramid means, VT ----
        g24x = poolb.tile([P, 4, 24, 24], BF16)
        v = gcolx.rearrange("p k (hb hi wb) -> p k hb hi wb", hi=4, wb=24)
        nc.vector.tensor_add(g24x, v[:, :, :, 0, :], v[:, :, :, 1, :])
        nc.vector.tensor_add(g24x, g24x, v[:, :, :, 2, :])
        nc.vector.tensor_add(g24x, g24x, v[:, :, :, 3, :])
        pfx = consts.tile([P, 4, S], BF16)
        _psp_stage2(nc, poolb, g24x, 4, pfx)

        vt_ps = psB.tile([P, NT], F32, tag="sim")
        for kc in range(4):
            nc.tensor.matmul(vt_ps[:S, :CV], pfx[:, kc, :], wvt[:, kc, :],
                             start=(kc == 0), stop=(kc == 3))
        vt = consts.tile([S, CV], BF16)
        nc.scalar.copy(vt, vt_ps[:S, :CV])

        kpx = consts.tile([P, 2, S], BF16)
        _psp_stage2(nc, poolb, g24k, 2, kpx)

        # ---- phase 3: attention + output, streamed over pairs of N tiles ----
        for tp in range(NTILES // 2):
            tt = (2 * tp, 2 * tp + 1)
            ens = []
            ctxb = work.tile([P, 2, 2, NT], BF16, tag="ctxb")  # [vc, tloc]
            for tloc, t in enumerate(tt):
                sim_ps = psB.tile([P, NT], F32, tag="sim")
                nc.tensor.matmul(sim_ps[:S, :], kpx[:, 0, :],
                                 kfb[:, 0, ts(t, NT)], start=True, stop=False)
                nc.tensor.matmul(sim_ps[:S, :], kpx[:, 1, :],
                                 kfb[:, 1, ts(t, NT)], start=False, stop=True)
                e1 = work.tile([P, NT], BF16, tag="e1")
                nc.scalar.activation(e1[:S, :], sim_ps[:S, :], EXP, scale=0.0625)
                cs_ps = psB.tile([1, NT], F32, tag="aux")
                nc.tensor.matmul(cs_ps, ones_col, e1[:S, :], start=True, stop=True)
                lrow = work.tile([1, NT], BF16, tag="lrow")
                nc.scalar.activation(lrow, cs_ps, LN)
                # rank-1: sim += (-16)*ln(colsum) => exp(sim/16) is normalized
                nc.tensor.matmul(sim_ps[:S, :], negs, lrow, start=False,
                                 stop=True, skip_group_check=True)
                en = work.tile([P, NT], BF16, tag="en")
                nc.scalar.activation(en[:S, :], sim_ps[:S, :], EXP, scale=0.0625)
                ens.append(en)
            for vc in range(2):
                ctx_ps = psB.tile([P, NT], F32, tag="aux")
                for tloc in range(2):
                    if tloc == 1:
                        ctx_ps = psB.tile([P, NT], F32, tag="aux")
                    nc.tensor.matmul(ctx_ps, vt[:, ts(vc, P)],
                                     ens[tloc][:S, :], start=True, stop=True)
                    nc.vector.tensor_copy(out=ctxb[:, vc, tloc, :], in_=ctx_ps)
            for mc in range(4):
                ops = psO.tile([P, 2, NT], F32, tag="outp")
                nc.tensor.matmul(ops[:, 0, :], ident, outb[:, mc, ts(tt[0], NT)],
                                 start=True, stop=False)
                nc.tensor.matmul(ops[:, 1, :], ident, outb[:, mc, ts(tt[1], NT)],
                                 start=True, stop=False)
                for vc in range(2):
                    last = vc == 1
                    nc.tensor.matmul(ops[:, 0, :], wct[:, vc, ts(mc, P)],
                                     ctxb[:, vc, 0, :], start=False, stop=last)
                    nc.tensor.matmul(ops[:, 1, :], wct[:, vc, ts(mc, P)],
                                     ctxb[:, vc, 1, :], start=False, stop=last)
                osb = outp.tile([P, 2, NT], F32, tag="osb")
                opsv = ops.rearrange("p j n -> p (j n)")
                osv = osb.rearrange("p j n -> p (j n)")
                if mc < 3:
                    nc.scalar.activation(osv, opsv, RELU, bias=bfb[:, mc:mc + 1])
                else:
                    nc.vector.tensor_scalar(osv, opsv, scalar1=bfb[:, mc:mc + 1],
                                            scalar2=0.0, op0=ADD, op1=MAX)
                nc.sync.dma_start(ov[:, mc, ds(2 * tp * NT, 2 * NT)], osv)


def _patch_ldw_opt():
    """walrus is invoked with --enable-ldw-opt=false; flip it so back-to-back
    matmuls sharing a stationary operand skip the redundant LDWEIGHTS (the
    fill/drain of consecutive matmuls then pipelines at the streaming rate)."""
    import concourse.bass_utils as bu

    if getattr(bu, "_apnb_ldw_patch", False):
        return
    orig = bu.run_command

    def patched(argv, **kw):
        argv = ["--enable-ldw-opt=true" if a == "--enable-ldw-opt=false" else a
                for a in argv]
        return orig(argv, **kw)

    bu.run_command = patched
    bu._apnb_ldw_patch = True


def _prune_redundant_ldweights(nc):
    """Remove back-to-back InstLdweights that reload the exact same stationary
    operand (walrus emits one LDWEIGHTS per matmul; our stationary-major loop
    order makes most of them redundant, and dropping them lets consecutive
    matmuls pipeline their fill/drain). All stationaries in this kernel are
    written exactly once before first use, so a signature match is sufficient.
    Pruned instructions' sync conditions are merged into the next PE
    instruction to preserve ordering.
    """
    import concourse.mybir as mybir

    def merge(a, b):
        if a is None:
            return b
        if b is None:
            return a
        waits = {}
        for w in list(a.on_wait) + list(b.on_wait):
            k = (w.sync_type, w.id, w.wait_mode)
            if k in waits:
                prev = waits[k]
                if (w.wait_value or 0) > (prev.wait_value or 0):
                    waits[k] = w
            else:
                waits[k] = w
        return mybir.SyncInfo(on_wait=list(waits.values()),
                              on_update=list(a.on_update) + list(b.on_update))

    n_pruned = 0
    for f in nc.m.functions:
        for blk in f.blocks:
            insts = list(blk.instructions)
            out = []
            last_sig = None
            pending = None
            for inst in insts:
                tname = type(inst).__name__
                eng = getattr(inst, "engine", None)
                if eng == mybir.EngineType.PE:
                    if tname == "InstLdweights":
                        ap = inst.ins[0]
                        sig = (ap.memref, ap.offset, str(ap.ap), str(ap.dtype),
                               str(inst.perf_mode), str(inst.is_transpose),
                               str(inst.tile_position), str(inst.tile_size))
                        if sig == last_sig and inst.sync_info is None:
                            n_pruned += 1
                            continue
                        last_sig = sig
                    elif tname in ("InstMatmult", "InstEventSemaphore"):
                        pass
                    else:
                        last_sig = None
                out.append(inst)
            blk.instructions = out
    return n_pruned


def _patch_act_tables():
    """Force every activation onto the one table that holds Exp, Ln, Relu and
    Copy together (`natural_log_exp_and_others`), so the kernel does a single
    ACT_TABLE_LOAD instead of reloading on every Exp<->Ln<->Relu switch.

    Table ids are positional (index into act_info.json), so we keep the dict
    order/size and just empty the other entries.
    """
    import concourse.hw_specs as hw_specs

    if getattr(bacc, "_apnb_act_patch", False):
        return
    orig = hw_specs.get_activation_tables

    def patched(module_arch):
        tabs = orig(module_arch)
        keep = "natural_log_exp_and_others"
        if keep not in tabs:
            return tabs
        return {k: (v if k == keep else set()) for k, v in tabs.items()}

    bacc.get_activation_tables = patched
    bacc._apnb_act_patch = True


def build_nc():
    _patch_act_tables()
    nc = bacc.Bacc("TRN2", target_bir_lowering=False, debug=False)
    x_d = nc.declare_dram_parameter("x", [CIN, N], BF16, isOutput=False)
    wkt_d = nc.declare_dram_parameter("wkt", [CIN, CK], BF16, isOutput=False)
    wvt_d = nc.declare_dram_parameter("wvt", [CIN, CV], BF16, isOutput=False)
    wct_d = nc.declare_dram_parameter("wct", [CV, COUT], BF16, isOutput=False)
    bwt_d = nc.declare_dram_parameter("bwt", [CIN, COUT], BF16, isOutput=False)
    bk_d = nc.declare_dram_parameter("bk", [CK], F32, isOutput=False)
    bf_d = nc.declare_dram_parameter("bf", [COUT], F32, isOutput=False)
    out_d = nc.declare_dram_parameter("out", [COUT, N], F32, isOutput=True)
    with tile.TileContext(nc) as tc:
        with ExitStack() as ctx:
            _build_body(ctx, tc, x_d, wkt_d, wvt_d, wct_d, bwt_d, bk_d, bf_d,
                        out_d)
    nc.compile()
    _prune_redundant_ldweights(nc)
    return nc


def _get_nc():
    global _NC
    if _NC is None:
        _NC = build_nc()
    return _NC


def fold_params(Wk, bk, gk, betak, mk, vk, Wv, bv, Ww, bw, Wo, bo, go, betao,
                mo, vo):
    """Fold BN params + the Ww conv into effective weights (all f32 numpy)."""
    bf16 = ml_dtypes.bfloat16
    sk = gk / np.sqrt(vk + EPS)
    Wk_f = sk[:, None] * Wk
    bk_f = (bk - mk) * sk + betak
    so = go / np.sqrt(vo + EPS)
    A = so[:, None] * Wo[:, :CIN]      # applies to ctx2 = Ww@ctx + bw
    Bw = so[:, None] * Wo[:, CIN:]     # applies to feats
    b0 = (bo - mo) * so + betao
    Wc = A @ Ww                        # (COUT, CV)
    # attn rows sum to 1  =>  value bias bv contributes Wc @ bv everywhere
    bf_ = b0 + A @ bw + Wc @ bv
    return {
        "wkt": np.ascontiguousarray(Wk_f.T).astype(bf16),
        "wvt": np.ascontiguousarray(Wv.T).astype(bf16),
        "wct": np.ascontiguousarray(Wc.T).astype(bf16),
        "bwt": np.ascontiguousarray(Bw.T).astype(bf16),
        "bk": bk_f.astype(np.float32),
        "bf": bf_.astype(np.float32),
    }


def kernel(**inputs):
    global LAST_RESULT
    feats = np.asarray(inputs["feats"], np.float32)
    B = feats.shape[0]
    assert feats.shape == (B, CIN, H, W) and B == N_CORES

    common = fold_params(
        np.asarray(inputs["Wk"], np.float32), np.asarray(inputs["bk"], np.float32),
        np.asarray(inputs["gk"], np.float32), np.asarray(inputs["betak"], np.float32),
        np.asarray(inputs["mk"], np.float32), np.asarray(inputs["vk"], np.float32),
        np.asarray(inputs["Wv"], np.float32), np.asarray(inputs["bv"], np.float32),
        np.asarray(inputs["Ww"], np.float32), np.asarray(inputs["bw"], np.float32),
        np.asarray(inputs["Wo"], np.float32), np.asarray(inputs["bo"], np.float32),
        np.asarray(inputs["go"], np.float32), np.asarray(inputs["betao"], np.float32),
        np.asarray(inputs["mo"], np.float32), np.asarray(inputs["vo"], np.float32),
    )
    bf16 = ml_dtypes.bfloat16
    in_maps = [
        {"x": np.ascontiguousarray(feats[i].reshape(CIN, N)).astype(bf16),
         **common}
        for i in range(N_CORES)
    ]
    nc = _get_nc()
    res = run_bass_kernel_spmd(nc, in_maps, core_ids=list(range(N_CORES)),
                               trace=PROFILE)
    LAST_RESULT = res
    out = np.stack([res.results[i]["out"].reshape(COUT, H, W)
                    for i in range(N_CORES)])
    return out.astype(np.float32)
